# revision 1
# baseline (speedup 1.0000x reference)
"""LIF neuron (STBP) forward kernel for Trainium2, 8-core data parallel.

Reference semantics (per element, scan over T):
    v = v * 0.9 + x_t
    s = (v >= 1.0)
    v = v - s * 1.0

Sharding: batch dim 32 -> 8 cores x 4. The recurrence is elementwise per
neuron, so no cross-core communication. Per core, all 4 local batches for
one timestep are fused into a single [128, 2048] SBUF tile (batch-major in
the free dim); the T loop keeps the membrane state v in SBUF and streams
x in / spikes out. Three VectorE ops per timestep:
    u = (v * beta) + x_t        scalar_tensor_tensor, 1x mode
    s = (u >= 1.0)              tensor_scalar is_ge,   2x mode
    v = u - s                   tensor_tensor sub,     1x mode
t=0 skips the first op (v0 = 0 so u = x_0) and computes per-batch so the
first spike tiles reach the store queue early; t=T-1 is also split
per-batch so the tail drains ASAP. DMA issue alternates between the two
HWDGE engines (sync, scalar) to halve per-engine dispatch serialization
and balance the two queue directions.
"""

from contextlib import ExitStack

import numpy as np

import concourse.bacc as bacc
import concourse.mybir as mybir
import concourse.tile as tile
from concourse.bass_utils import run_bass_kernel_spmd

N_CORES = 8
B, T, C, H, W = 32, 16, 64, 32, 32
B_LOC = B // N_CORES  # 4 batches per core
P = 128               # SBUF partitions
F = (C * H * W) // P  # 512 free elements per partition per batch
FB = B_LOC * F        # 2048 free elements in a fused all-batch tile
BETA = 0.9
V_TH = 1.0

_CACHE = {}


def _build(repeat: int = 1, loop_n: int = 1):
    """Build + compile the per-core Bass program (identical on all cores).

    repeat > 1 unrolls the whole (idempotent) kernel body that many times
    inside one NEFF; loop_n > 1 additionally wraps it in a hardware loop.
    Both are used only for wall-clock timing of the device step.
    """
    nc = bacc.Bacc(
        "TRN2", target_bir_lowering=False, debug=False, num_devices=N_CORES
    )
    x = nc.dram_tensor(
        "x", [B_LOC, T, P, F], mybir.dt.float32, kind="ExternalInput"
    ).ap()
    s_out = nc.dram_tensor(
        "s", [B_LOC, T, P, F], mybir.dt.float32, kind="ExternalOutput"
    ).ap()

    with tile.TileContext(nc) as tc:
        def emit_body():
            _emit(nc, tc, x, s_out, repeat)

        if loop_n > 1:
            with tc.For_i(
                0, loop_n, 1,
                hint_engines=(
                    mybir.EngineType.SP,
                    mybir.EngineType.Activation,
                    mybir.EngineType.DVE,
                ),
            ):
                emit_body()
        else:
            emit_body()

    nc.compile()
    return nc


def _emit(nc, tc, x, s_out, repeat):
    with ExitStack() as ctx:
        xp = ctx.enter_context(tc.tile_pool(name="xp", bufs=6))
        up = ctx.enter_context(tc.tile_pool(name="up", bufs=2))
        sp = ctx.enter_context(tc.tile_pool(name="sp", bufs=4))
        vp = ctx.enter_context(tc.tile_pool(name="vp", bufs=2))

        # Alternate HWDGE issuing engine per (t, b) so each engine carries
        # half the inputs and half the outputs.
        def in_eng(t, b):
            return nc.sync if (t * B_LOC + b) % 2 == 0 else nc.scalar

        def out_eng(t, b):
            return nc.scalar if (t * B_LOC + b) % 2 == 0 else nc.sync

        v = None
        for t in [t for _ in range(repeat) for t in range(T)]:
            xt = xp.tile([P, FB], mybir.dt.float32)
            for b in range(B_LOC):
                in_eng(t, b).dma_start(xt[:, b * F:(b + 1) * F], x[b, t])

            st = sp.tile([P, FB], mybir.dt.float32)

            if t == 0:
                # v0 = 0 -> u = x0; compute per-batch so spikes for batch b
                # are ready as soon as its input lands.
                vn = vp.tile([P, FB], mybir.dt.float32)
                for b in range(B_LOC):
                    sl = slice(b * F, (b + 1) * F)
                    nc.vector.tensor_scalar(
                        st[:, sl], xt[:, sl], V_TH, None, mybir.AluOpType.is_ge
                    )
                    out_eng(t, b).dma_start(s_out[b, t], st[:, sl])
                    nc.vector.tensor_sub(vn[:, sl], xt[:, sl], st[:, sl])
                v = vn
                continue

            u = up.tile([P, FB], mybir.dt.float32)
            if t < T - 1:
                nc.vector.scalar_tensor_tensor(
                    u[:], v[:], BETA, xt[:],
                    mybir.AluOpType.mult, mybir.AluOpType.add,
                )
                nc.vector.tensor_scalar(
                    st[:], u[:], V_TH, None, mybir.AluOpType.is_ge
                )
                for b in range(B_LOC):
                    out_eng(t, b).dma_start(s_out[b, t], st[:, b * F:(b + 1) * F])
                vn = vp.tile([P, FB], mybir.dt.float32)
                nc.vector.tensor_sub(vn[:], u[:], st[:])
                v = vn
            else:
                # Last step: no v update needed; split per-batch so the
                # final stores start draining before all compute finishes.
                for b in range(B_LOC):
                    sl = slice(b * F, (b + 1) * F)
                    nc.vector.scalar_tensor_tensor(
                        u[:, sl], v[:, sl], BETA, xt[:, sl],
                        mybir.AluOpType.mult, mybir.AluOpType.add,
                    )
                    nc.vector.tensor_scalar(
                        st[:, sl], u[:, sl], V_TH, None, mybir.AluOpType.is_ge
                    )
                    out_eng(t, b).dma_start(s_out[b, t], st[:, sl])


def _get_nc(repeat: int = 1, loop_n: int = 1):
    key = f"nc{repeat}_{loop_n}"
    if key not in _CACHE:
        _CACHE[key] = _build(repeat, loop_n)
    return _CACHE[key]


def _run(x_seq: np.ndarray, trace: bool = False, repeat: int = 1):
    """Shard, execute on 8 cores, gather. Returns (output, BassKernelResults)."""
    nc = _get_nc(repeat)
    x_seq = np.ascontiguousarray(x_seq, dtype=np.float32)
    in_maps = [
        {"x": x_seq[i * B_LOC:(i + 1) * B_LOC].reshape(B_LOC, T, P, F)}
        for i in range(N_CORES)
    ]
    res = run_bass_kernel_spmd(
        nc, in_maps, core_ids=list(range(N_CORES)), trace=trace
    )
    out = np.concatenate(
        [r["s"].reshape(B_LOC, T, C, H, W) for r in res.results], axis=0
    )
    return out, res


def kernel(x_seq: np.ndarray) -> np.ndarray:
    out, _ = _run(x_seq, trace=False)
    return out

